# revision 1
# baseline (speedup 1.0000x reference)
"""Multi-head attention (B=2, S=2048, D=1024, H=16, causal) on 8 TRN2 NeuronCores.

Sharding: 8 cores = 2 batches x 4 head-groups (4 heads each).  Each core
computes the QKV projections for its head slice, causal attention for its 4
heads, and the partial output projection (input-dim slice of Wo).  The
all-reduce over head-groups happens at gather time on the host (sum of 4
partials per batch).

Everything on device works in token-transposed layout ([feature, token]):
  scores^T[kv, q] = K_projT_tile^T @ Q_projT   (K = dh = 64)
  P = exp(scores^T)  (no max subtraction: scores ~ N(0,1), |s| < ~7)
  out^T[dh(+1), q] = [V | ones]^T @ P          (ones column -> softmax denom)
  partial^T[dmodel, tok] = WoT_slice^T @ attn_out^T

v2 restructure (from trace analysis of the 227us baseline):
  - inputs stream in 512-column groups (q_n, k_n, v_n) ordered by need time;
    projections and attention chase the stream so the PE goes dense at ~5us
    and the HAM clock never re-throttles (was: 28us at half clock).
  - score pairs land in one 2-bank PSUM tile [128, 2, 512]; ONE exp
    activation covers both heads of a pair (halves ACT instruction count).
  - softmax normalize: denominator row -> 1-lane reciprocal -> gpsimd
    partition_broadcast (was: two DMA round-trips through a [128,4] tile).
  - output partials in bf16 (halves output DMA), summed in f32 on host.
  - attention emitted j-ascending with oproj(j) right after attn(1,j); the
    attn unit loop interleaves with projection groups of the next segment.
"""

import math
import os

import numpy as np
import ml_dtypes

_BF16 = ml_dtypes.bfloat16

B, S, D = 2, 2048, 1024
H, DH = 16, 64
NCORES = 8
GRP = 4  # heads per core
KT = D // 128  # 8 k-tiles over d_model
NQ = 512  # q tile width
QTILES = S // NQ  # 4
KVTILES = S // 128  # 16

last_results = None

_programs = {}


def _build_program(causal: bool):
    import concourse.bass as bass
    import concourse.mybir as mybir
    import concourse.tile as tile
    from concourse import bacc

    f32 = mybir.dt.float32
    bf16 = mybir.dt.bfloat16
    Exp = mybir.ActivationFunctionType.Exp

    nc = bacc.Bacc(
        "TRN2",
        target_bir_lowering=False,
        debug=False,
        enable_asserts=False,
        num_devices=NCORES,
    )

    # all inputs host-pre-tiled so every DMA is one instruction with >=8KB
    # contiguous per-partition lines (1KB lines are packet-bound at ~97GB/s)
    qT = nc.dram_tensor("qT", [QTILES, 128, KT * NQ], bf16, kind="ExternalInput").ap()
    kT = nc.dram_tensor("kT", [QTILES, 128, KT * NQ], bf16, kind="ExternalInput").ap()
    vT = nc.dram_tensor("vT", [QTILES, 128, KT * NQ], bf16, kind="ExternalInput").ap()
    wqT = nc.dram_tensor("wqT", [128, KT * 256], bf16, kind="ExternalInput").ap()
    wkT = nc.dram_tensor("wkT", [128, KT * 256], bf16, kind="ExternalInput").ap()
    wvT = nc.dram_tensor("wvT", [128, KT * 256], bf16, kind="ExternalInput").ap()
    woT = nc.dram_tensor("woT", [128, 2 * D], bf16, kind="ExternalInput").ap()
    if not causal:
        maskT = nc.dram_tensor("maskT", [S, S], bf16, kind="ExternalInput").ap()
    out = nc.dram_tensor("out", [D, S], bf16, kind="ExternalOutput").ap()

    with tile.TileContext(nc) as tc:
        with (
            tc.tile_pool(name="persist", bufs=1) as sb,
            tc.tile_pool(name="stream", bufs=3) as stream,
            tc.tile_pool(name="psum", bufs=1, space="PSUM") as psum,
            tc.tile_pool(name="p_sb", bufs=6) as pbuf,
            tc.tile_pool(name="r_sb", bufs=2) as rpool,
            tc.tile_pool(name="m_sb", bufs=4) as mpool,
            tc.tile_pool(name="o_sb", bufs=4) as opool,
        ):
            # ---- persistent SBUF tensors ----
            wq_sb = sb.tile([128, KT, 256], bf16)
            wk_sb = sb.tile([128, KT, 256], bf16)
            wv_sb = sb.tile([128, KT, 256], bf16)
            wo2 = sb.tile([128, 2, D], bf16)  # head h at rows 64*(h%2), chunk h//2
            qproj = sb.tile([128, 2, S], bf16)
            kproj = sb.tile([128, 2, S], bf16)
            attn2 = sb.tile([128, 2, S], bf16)  # head h at rows 64*(h%2), chunk h//2

            # ---- input DMA prologue: everything issued up front, ordered by
            # need time.  sync (HWDGE): wq, q/v groups, wo.  gpsimd (SWDGE):
            # wk, wv, k groups.  Stream tiles are [128, KT, 512] (one group =
            # 512 token-columns of all 8 k-tiles, one DMA instruction each).
            # three DMA rings: sync (SP HWDGE) carries q + wo + late v;
            # scalar (Act HWDGE) carries v + late k; gpsimd (SWDGE) carries
            # early k.  The scalar-ring DMA instructions all issue before the
            # first exp, so they cost ~3us of ACT queue time up front.
            nc.sync.dma_start(wq_sb[:], wqT[:])
            nc.gpsimd.dma_start(wk_sb[:], wkT[:])

            # gpsimd setup work must precede the k-group DMA instructions on
            # the gpsimd FIFO (k3's buffer-reuse wait would delay it ~15us)
            vproj = sb.tile([128, KVTILES, GRP, 66], bf16)
            # ones columns at index 0 and 65 of vproj (V lands in cols 1..64)
            nc.gpsimd.memset(vproj[:], 1.0)
            # PE warmup: dummy matmuls fill the pre-input idle window and the
            # DMA-paced projection phase so the HAM clock gate stays open.
            warm = sb.tile([128, 256], bf16)
            nc.gpsimd.memset(warm[:], 0.0)
            _warm_ctr = [0]

            def warmup(k):
                for _ in range(k):
                    w = _warm_ctr[0]
                    _warm_ctr[0] += 1
                    wp = psum.tile(
                        [128, 256], f32, tag="mm", bufs=2, name=f"warm{w}"
                    )
                    nc.tensor.matmul(
                        wp[:], warm[:, 0:128], warm[:], start=True, stop=True
                    )

            warmup(24)
            if causal:
                # single 128x128 causal block: keep where q_local >= kv_local
                mask128 = sb.tile([128, 128], bf16)
                nc.gpsimd.memset(mask128[:], 1.0)
                nc.gpsimd.affine_select(
                    out=mask128[:],
                    in_=mask128[:],
                    compare_op=mybir.AluOpType.is_ge,
                    fill=0.0,
                    base=0,
                    pattern=[[1, 128]],
                    channel_multiplier=-1,
                )

            qg, kg, vg = [], [], []

            def _group(dst_list, src, tag, eng, bufs=3):
                n = len(dst_list)
                t = stream.tile(
                    [128, KT, NQ], bf16, tag=tag, bufs=bufs, name=f"{tag}{n}"
                )
                eng.dma_start(t[:], src[n])
                dst_list.append(t)

            # sync: q0, v0, q1, v1, q2, q3, wo, v2, v3
            _group(qg, qT, "qg", nc.sync)
            _group(vg, vT, "vg", nc.sync)
            _group(qg, qT, "qg", nc.sync)
            _group(vg, vT, "vg", nc.sync)
            _group(qg, qT, "qg", nc.sync)
            _group(qg, qT, "qg", nc.sync)
            nc.sync.dma_start(wo2[:], woT[:])
            _group(vg, vT, "vg", nc.sync)
            _group(vg, vT, "vg", nc.sync)
            # gpsimd: k0 first (scores j=0 gate ACT start), wv, then k1..k3
            _group(kg, kT, "kg", nc.gpsimd)
            nc.gpsimd.dma_start(wv_sb[:], wvT[:])
            for n in range(1, QTILES):
                _group(kg, kT, "kg", nc.gpsimd)

            def qkproj(which, m2, n):
                w_sb = wq_sb if which == "q" else wk_sb
                xt = (qg if which == "q" else kg)[n]
                proj = qproj if which == "q" else kproj
                ps = psum.tile([128, NQ], f32, tag="mm", bufs=2)
                for kt in range(KT):
                    nc.tensor.matmul(
                        ps[:],
                        w_sb[:, kt, 128 * m2 : 128 * m2 + 128],
                        xt[:, kt, :],
                        start=(kt == 0),
                        stop=(kt == KT - 1),
                    )
                nc.vector.tensor_copy(proj[:, m2, NQ * n : NQ * n + NQ], ps[:])

            def vproj_tile(mt):
                vt = vg[mt // 4]
                col = 128 * (mt % 4)
                ps = psum.tile([128, 256], f32, tag="mm", bufs=2)
                for kt in range(KT):
                    nc.tensor.matmul(
                        ps[:],
                        vt[:, kt, col : col + 128],
                        wv_sb[:, kt, :],
                        start=(kt == 0),
                        stop=(kt == KT - 1),
                    )
                nc.vector.tensor_copy(
                    vproj[:, mt, :, 1:65],
                    ps[:].rearrange("p (h d) -> p h d", h=GRP),
                )

            # attention state per (c2): av psum tile + per-tile units
            def attn_units(c2, j, avt):
                """Yield per-kv-tile unit emitters for head pair c2, q-tile j."""
                ktiles = 4 * j + 4 if causal else KVTILES
                p2s = {}

                def sc_exp(t):
                    d = t - 4 * j
                    off = 128 * d if (causal and d >= 0) else 0
                    sc = psum.tile([128, 2, NQ], f32, tag="sc", bufs=2)
                    for i in range(2):
                        base = 64 * i
                        nc.tensor.matmul(
                            sc[:, i, off:NQ],
                            kproj[base : base + 64, c2, 128 * t : 128 * t + 128],
                            qproj[base : base + 64, c2, NQ * j + off : NQ * j + NQ],
                            start=True,
                            stop=True,
                        )
                    p2 = pbuf.tile([128, 2, NQ], bf16, tag="p")
                    p2s[t] = p2
                    nc.scalar.activation(p2[:, :, off:NQ], sc[:, :, off:NQ], Exp)

                def mask_av(t):
                    d = t - 4 * j
                    off = 128 * d if (causal and d >= 0) else 0
                    p2 = p2s[t]
                    if causal:
                        if d >= 0:
                            for i in range(2):
                                nc.vector.tensor_mul(
                                    p2[:, i, off : off + 128],
                                    p2[:, i, off : off + 128],
                                    mask128[:],
                                )
                    else:
                        mt_t = mpool.tile([128, NQ], bf16, tag="mt")
                        nc.sync.dma_start(
                            mt_t[:],
                            maskT[128 * t : 128 * t + 128, NQ * j : NQ * j + NQ],
                        )
                        for i in range(2):
                            nc.vector.tensor_mul(p2[:, i, :], p2[:, i, :], mt_t[:])
                    for i in range(2):
                        nc.tensor.matmul(
                            avt[:, i, off:NQ],
                            vproj[:, t, 2 * c2 + i, 1:66],
                            p2[:, i, off:NQ],
                            start=(t == 0),
                            stop=(t == ktiles - 1),
                        )

                def unit(t):
                    sc_exp(t)
                    mask_av(t)

                return ktiles, sc_exp, mask_av, unit

            def normalize(c2, j, avt):
                # av rows 0..63 = V @ P, row 64 = denominator (ones column).
                # One copy releases the av PSUM banks immediately; the rest of
                # the chain runs from SBUF so the next pair's AV can start.
                avs = rpool.tile([65, 2, NQ], f32, tag="avs")
                nc.vector.tensor_copy(avs[:], avt[:])
                # reciprocal is ~6.4ns/elem/lane: spread the 1024 denominators
                # over all 128 lanes via a DMA reshape (and land the result on
                # partition 0, where partition_broadcast reads).
                rq = rpool.tile([128, 8], f32, tag="rq")
                nc.gpsimd.dma_start(rq[:], avs[64:65, :, :])
                rqr = rpool.tile([128, 8], f32, tag="rqr")
                nc.vector.reciprocal(rqr[:], rq[:])
                rz = rpool.tile([1, 2, NQ], f32, tag="rz")
                nc.gpsimd.dma_start(rz[:], rqr[:])
                rb = rpool.tile([64, 2, NQ], f32, tag="rb")
                for i in range(2):
                    nc.gpsimd.partition_broadcast(
                        rb[0:64, i, :], rz[0:1, i, :], channels=64
                    )
                nc.vector.tensor_mul(
                    attn2[0:64, c2, NQ * j : NQ * j + NQ], avs[0:64, 0, :], rb[:, 0, :]
                )
                tmpn = rpool.tile([64, NQ], bf16, tag="tmpn")
                nc.vector.tensor_mul(tmpn[:], avs[0:64, 1, :], rb[:, 1, :])
                nc.gpsimd.dma_start(attn2[64:128, c2, NQ * j : NQ * j + NQ], tmpn[:])

            def attn_pair(c2, j, fillers=(), split=False, mid_fillers=(),
                          tail_fillers=()):
                """Emit one head-pair x q-tile attention, interleaving any
                filler emitters (projection groups) between kv-tile units.
                split=True emits all scores+exp first (with `fillers`
                interleaved), then `mid_fillers`, then all mask+AV passes --
                used when the AV inputs (vproj) are not ready yet."""
                avt = psum.tile([65, 2, NQ], f32, tag="av", bufs=1, name=f"av{c2}{j}")
                ktiles, sc_exp, mask_av, unit = attn_units(c2, j, avt)

                def spread(emitters, fill):
                    nf, nu = len(fill), len(emitters)
                    fi = 0
                    for ui, u in enumerate(emitters):
                        u()
                        while fi < nf and fi * nu <= (ui + 1) * nf - 1:
                            fill[fi]()
                            fi += 1
                    while fi < nf:
                        fill[fi]()
                        fi += 1

                if split:
                    spread([lambda t=t: sc_exp(t) for t in range(ktiles)],
                           list(fillers))
                    for g in mid_fillers:
                        g()
                    spread([lambda t=t: mask_av(t) for t in range(ktiles)],
                           list(tail_fillers))
                else:
                    spread([lambda t=t: unit(t) for t in range(ktiles)],
                           list(fillers))
                normalize(c2, j, avt)

            def oproj_m(n, m):
                ps = psum.tile([128, NQ], f32, tag="mm", bufs=2)
                for c2 in range(2):
                    nc.tensor.matmul(
                        ps[:],
                        wo2[:, c2, 128 * m : 128 * m + 128],
                        attn2[:, c2, NQ * n : NQ * n + NQ],
                        start=(c2 == 0),
                        stop=(c2 == 1),
                    )
                ot = opool.tile([128, NQ], bf16, tag="ot")
                nc.vector.tensor_copy(ot[:], ps[:])
                nc.sync.dma_start(
                    out[128 * m : 128 * m + 128, NQ * n : NQ * n + NQ], ot[:]
                )

            def oproj_n(n):
                for m in range(D // 128):
                    oproj_m(n, m)

            def Q(m2, n):
                return lambda: qkproj("q", m2, n)

            def K_(m2, n):
                return lambda: qkproj("k", m2, n)

            def V2(n, half):
                mts = range(4 * n + 2 * half, 4 * n + 2 * half + 2)
                def f():
                    for mt in mts:
                        vproj_tile(mt)
                return f

            def OP(n, ms):
                def f():
                    for m in ms:
                        oproj_m(n, m)
                return f

            # ---- emission schedule ----
            # PE is the binding engine: keep it dense.  Fillers are spread
            # between attention units; a pair's fillers must not be among its
            # own dependencies EXCEPT vproj fillers, which land earlier in the
            # unit loop than the first unit that reads them (verified against
            # the spread formula).  The small (1,0) pair runs last to keep the
            # post-exp tail short.
            for g in [Q(0, 0), Q(1, 0), K_(0, 0), K_(1, 0), V2(0, 0), V2(0, 1)]:
                g()
            attn_pair(0, 0, fillers=[Q(0, 1), K_(0, 1), Q(1, 1), K_(1, 1)])
            attn_pair(0, 1, fillers=[V2(1, 0), V2(1, 1), Q(0, 2), K_(0, 2)])
            attn_pair(1, 1, fillers=[Q(1, 2), K_(1, 2), Q(0, 3), K_(0, 3)])
            attn_pair(0, 2, fillers=[V2(2, 0), V2(2, 1), Q(1, 3), K_(1, 3)])
            attn_pair(1, 2, fillers=[OP(1, range(0, 2)), OP(1, range(2, 4)),
                                     OP(1, range(4, 6)), OP(1, range(6, 8))])
            attn_pair(0, 3, fillers=[V2(3, 0), V2(3, 1), OP(2, range(0, 3)),
                                     OP(2, range(3, 6))])
            attn_pair(1, 3, fillers=[OP(2, range(6, 8))])
            attn_pair(1, 0, fillers=[OP(3, range(0, 4)), OP(3, range(4, 8))])
            oproj_n(0)

    nc.compile()
    return nc


def _get_program(causal: bool):
    if causal not in _programs:
        _programs[causal] = _build_program(causal)
    return _programs[causal]


def kernel(query, key, value, mask, Wq, Wk, Wv, Wo):
    global last_results
    from concourse.bass_utils import run_bass_kernel_spmd

    query = np.asarray(query, dtype=np.float32)
    key = np.asarray(key, dtype=np.float32)
    value = np.asarray(value, dtype=np.float32)
    Wq = np.asarray(Wq, dtype=np.float32)
    Wk = np.asarray(Wk, dtype=np.float32)
    Wv = np.asarray(Wv, dtype=np.float32)
    Wo = np.asarray(Wo, dtype=np.float32)
    m2d = np.asarray(mask).reshape(S, S).astype(bool)

    causal = bool(np.array_equal(m2d, np.tril(np.ones((S, S), dtype=bool))))
    nc = _get_program(causal)

    scale = 1.0 / math.sqrt(DH)
    WqT = np.ascontiguousarray((Wq * scale).T).astype(_BF16)
    WkT = np.ascontiguousarray(Wk.T).astype(_BF16)
    WvT = np.ascontiguousarray(Wv.T).astype(_BF16)
    WoT = np.ascontiguousarray(Wo.T).astype(_BF16)

    def tile_x(xTb):
        # [D, S] -> [QTILES, 128, KT*512]: group n holds token-columns
        # [512n, 512n+512) of all KT row-tiles, 8KB contiguous per partition
        return np.ascontiguousarray(
            xTb.reshape(KT, 128, QTILES, NQ).transpose(2, 1, 0, 3).reshape(
                QTILES, 128, KT * NQ
            )
        )

    def tile_w(wT):
        # [D, 256] -> [128, KT*256]
        return np.ascontiguousarray(
            wT.reshape(KT, 128, 256).transpose(1, 0, 2).reshape(128, KT * 256)
        )

    def tile_wo(woTs):
        # [256, D] -> [128, 2*D]: head h rows at 64*(h%2), chunk h//2
        o = np.zeros((128, 2, D), dtype=woTs.dtype)
        for h in range(GRP):
            base = 64 * (h % 2)
            o[base : base + 64, h // 2, :] = woTs[64 * h : 64 * h + 64, :]
        return np.ascontiguousarray(o.reshape(128, 2 * D))

    xT = {
        "qT": [tile_x(query[b].T.astype(_BF16)) for b in range(B)],
        "kT": [tile_x(key[b].T.astype(_BF16)) for b in range(B)],
        "vT": [tile_x(value[b].T.astype(_BF16)) for b in range(B)],
    }
    if not causal:
        maskT = np.ascontiguousarray(m2d.T).astype(_BF16)

    in_maps = []
    for c in range(NCORES):
        b, g = c // 4, c % 4
        sl = slice(256 * g, 256 * g + 256)
        im = {
            "qT": xT["qT"][b],
            "kT": xT["kT"][b],
            "vT": xT["vT"][b],
            "wqT": tile_w(WqT[:, sl]),
            "wkT": tile_w(WkT[:, sl]),
            "wvT": tile_w(WvT[:, sl]),
            "woT": tile_wo(WoT[sl, :]),
        }
        if not causal:
            im["maskT"] = maskT
        in_maps.append(im)

    trace = os.environ.get("KERNEL_PROFILE", "") == "1"
    res = run_bass_kernel_spmd(nc, in_maps, list(range(NCORES)), trace=trace)
    last_results = res

    outp = np.empty((B, S, D), dtype=np.float32)
    for b in range(B):
        acc = res.results[4 * b]["out"].astype(np.float32)
        for g in range(1, 4):
            acc = acc + res.results[4 * b + g]["out"].astype(np.float32)
        outp[b] = acc.T
    return outp

